# revision 32
# baseline (speedup 1.0000x reference)
"""Trainium2 Bass kernel for nn_CodeLinearAttention (B=2, T=2048, D=1024,
H=16, HD=64, C=16) on 8 NeuronCores.

Sharding: core c -> batch b = c//4, head group g = c%4 (4 heads per core).

v2 design (bf16 matmuls, transposed-native layouts):
  - The code projection (16x64 per head) is folded into the qkv weights on
    the host: w_eff_q[h] = code_h @ Wq_h, so the kernel computes
    qc^T/kc^T = w_eff @ x^T directly in [c, t] layout ([128, T] tiles with
    head h's 16 c-rows at partition 32h; rows 32h+16..32h+32 unused).
  - eq = exp(qc*scale), ek = exp(kc*scale) via one Act op per 512-token
    tile (exp IS the PSUM->SBUF copy). Reference's max-subtractions cancel
    identically (up to the 1e-9 eps) and are dropped.
  - Cumsum of ek over t per chunk of 128: transpose ek chunk to [t, hc]
    (PE transpose), then kcumT = ek_sc^T @ triu. The running carry AND the
    1e-9 eps live in a per-partition bias vector applied by the Act engine
    (rt = kcum + bias); bias += kcum[:, -1] per chunk (DVE).
  - qn = eq * (1/rt) * (1/S broadcast): S_q summed over c via a 0/1
    selection matmul (broadcast back over (c) rows in the same matmul),
    so xo needs no further normalization.
  - Chunked causal linear attention (chunk=128): at = ek_h^T qn_h (K=16)
    into one [128, 4*128] PSUM tile; masked+bf16 via one DVE multiply with
    a 4x-tiled triu; xo per head-pair accumulates v-term and kv-term in a
    [64, 512] PSUM tile (head in col halves, 2 chunks per group); KV state
    via one [128, 256] cross-head outer-product matmul per chunk (only the
    diagonal head blocks are consumed), accumulated in SBUF fp32 with a
    bf16 shadow copied on the (otherwise idle) GpSimd engine.
  - Output projection per 2-chunk group; fp16 staging tiles DMA'd to the
    partial output; host sums the 4 partials of each batch.
The post-softmax *scale is folded into w_outT on the host (exact pow2).
"""

import sys

sys.path.insert(0, "/opt/trn_rl_repo")

from contextlib import ExitStack

import numpy as np
import ml_dtypes

import concourse.bacc as bacc
import concourse.tile as tile
from concourse import mybir

F32 = mybir.dt.float32
F16 = mybir.dt.float16
BF16 = mybir.dt.bfloat16
AF = mybir.ActivationFunctionType
OP = mybir.AluOpType

B, T, D, NHEAD, HD, C = 2, 2048, 1024, 16, 64, 16
HC = 4  # heads per core
CH = 128  # attention chunk
NCH = T // CH  # 16
TC = 512  # projection t-chunk
NTC = T // TC  # 4
SCALE = HD ** -0.5  # 0.125
N_CORES = 8
BFNP = ml_dtypes.bfloat16


def emit_body(nc, tc_, ctx, io):
    xT, wqkT, wvT, woutT, triu4, iden128, selhc, out = io

    const = ctx.enter_context(tc_.tile_pool(name="const", bufs=1))
    persist = ctx.enter_context(tc_.tile_pool(name="persist", bufs=1))
    xin = ctx.enter_context(tc_.tile_pool(name="xin", bufs=10))
    atp = ctx.enter_context(tc_.tile_pool(name="atp", bufs=2))
    xop = ctx.enter_context(tc_.tile_pool(name="xop", bufs=2))
    sip = ctx.enter_context(tc_.tile_pool(name="sip", bufs=2))
    rtp = ctx.enter_context(tc_.tile_pool(name="rtp", bufs=2))
    kvp = ctx.enter_context(tc_.tile_pool(name="kvp", bufs=2))
    qnp = ctx.enter_context(tc_.tile_pool(name="qnp", bufs=2))
    bip = ctx.enter_context(tc_.tile_pool(name="bip", bufs=2))
    ostg = ctx.enter_context(tc_.tile_pool(name="ostg", bufs=3))

    psA = ctx.enter_context(tc_.tile_pool(name="psA", bufs=2, space="PSUM"))
    psB = ctx.enter_context(tc_.tile_pool(name="psB", bufs=2, space="PSUM"))
    psXO = ctx.enter_context(tc_.tile_pool(name="psXO", bufs=1, space="PSUM"))
    psSm = ctx.enter_context(tc_.tile_pool(name="psSm", bufs=2, space="PSUM"))

    # ---- constants / weights in SBUF (weight DMAs on the Act queue) ----
    wqk_sb = []
    for dci in range(8):
        w = const.tile([128, 256], BF16, tag=f"wqk{dci}", name=f"wqk{dci}")
        nc.scalar.dma_start(w[:], wqkT[dci * 128 : (dci + 1) * 128, :])
        wqk_sb.append(w)
    wv_sb = []
    for dci in range(8):
        w = const.tile([128, 256], BF16, tag=f"wv{dci}", name=f"wv{dci}")
        nc.scalar.dma_start(w[:], wvT[dci * 128 : (dci + 1) * 128, :])
        wv_sb.append(w)
    wout_sb = []
    for j in range(HC):
        w = const.tile([64, 1024], BF16, tag=f"wout{j}", name=f"wout{j}")
        nc.scalar.dma_start(w[:], woutT[j * 64 : (j + 1) * 64, :])
        wout_sb.append(w)
    triu4_sb = const.tile([128, 512], BF16)
    nc.scalar.dma_start(triu4_sb[:], triu4)
    iden_sb = const.tile([128, 128], BF16)
    nc.scalar.dma_start(iden_sb[:], iden128)
    sel_sb = const.tile([128, 128], BF16)
    nc.scalar.dma_start(sel_sb[:], selhc)

    # ---- persistent SBUF tensors ----
    eqT = persist.tile([128, T], BF16, tag="eqT")  # head h c-rows at 32h
    ekT = persist.tile([128, T], BF16, tag="ekT")
    qnT = persist.tile([128, T], BF16, tag="qnT")
    v_sb = persist.tile([128, NCH * 256], BF16, tag="v_sb")  # [t, (h,d)]
    eksc = persist.tile([128, NCH * 128], BF16, tag="eksc")  # [t, hc] per chunk
    ek4 = persist.tile([16, HC * T], BF16, tag="ek4")  # compact [c, (h, t)]
    kvf = persist.tile([16, 256], F32, tag="kvf")  # [c, (h, d)]
    nc.vector.memset(kvf[:], 0.0)

    # ================= P1: projections =================
    for tci in range(NTC):
        tsl5 = slice(tci * TC, (tci + 1) * TC)
        xts = []
        for dci in range(8):
            xt = xin.tile([128, TC], BF16, tag="xt", name="xt")
            nc.sync.dma_start(xt[:], xT[dci * 128 : (dci + 1) * 128, tsl5])
            xts.append(xt)
        q_ps = psA.tile([128, TC], F32, tag="psA", name="q_ps")
        for dci in range(8):
            nc.tensor.matmul(
                q_ps[:], lhsT=wqk_sb[dci][:, 0:128], rhs=xts[dci][:],
                start=(dci == 0), stop=(dci == 7),
            )
        nc.scalar.activation(eqT[:, tsl5], q_ps[:], AF.Exp, scale=SCALE)
        k_ps = psA.tile([128, TC], F32, tag="psA", name="k_ps")
        for dci in range(8):
            nc.tensor.matmul(
                k_ps[:], lhsT=wqk_sb[dci][:, 128:256], rhs=xts[dci][:],
                start=(dci == 0), stop=(dci == 7),
            )
        nc.scalar.activation(ekT[:, tsl5], k_ps[:], AF.Exp, scale=SCALE)
        for j in range(HC):
            nc.scalar.dma_start(
                ek4[0:16, j * T + tci * TC : j * T + (tci + 1) * TC],
                ekT[32 * j : 32 * j + 16, tsl5],
            )
        for sub in range(4):
            v_ps = psB.tile([128, 256], F32, tag="psB", name="v_ps")
            for dci in range(8):
                nc.tensor.matmul(
                    v_ps[:],
                    lhsT=xts[dci][:, sub * 128 : (sub + 1) * 128],
                    rhs=wv_sb[dci][:],
                    start=(dci == 0), stop=(dci == 7),
                )
            ci = tci * 4 + sub
            if sub % 2 == 0:
                nc.vector.tensor_copy(
                    out=v_sb[:, ci * 256 : (ci + 1) * 256], in_=v_ps[:]
                )
            else:
                nc.scalar.copy(v_sb[:, ci * 256 : (ci + 1) * 256], v_ps[:])

    # ============ P2+P3 fused per-chunk loop ============
    bias = bip.tile([128, 1], F32, tag="bias", name="bias0")
    nc.vector.memset(bias[:], 1e-9)
    kvb = None
    xo_ps = [None, None]
    xoTs = [None, None]
    Sinv = None
    for i in range(NCH):
        tsl = slice(i * CH, (i + 1) * CH)
        g2, slot2 = i // 2, i % 2
        g4, slot4 = i // 4, i % 4
        # --- S over 4-chunk groups: S[hc, t] broadcast over c rows ---
        if slot4 == 0:
            gsl4 = slice(g4 * 512, (g4 + 1) * 512)
            sf_ps = psSm.tile([128, 512], F32, tag="sm", name="sf_ps")
            nc.tensor.matmul(
                sf_ps[:], lhsT=sel_sb[:], rhs=eqT[:, gsl4], start=True, stop=True
            )
            Sinv = sip.tile([128, 512], BF16, tag="Sinv", name="Sinv")
            nc.vector.reciprocal(Sinv[:], sf_ps[:])
        if slot2 == 0:
            xo_ps[0] = psXO.tile([64, 512], F32, tag="xo0", name="xo0")
            xo_ps[1] = psXO.tile([64, 512], F32, tag="xo1", name="xo1")
        # --- transpose ek chunk to [t, hc] ---
        et_ps = psSm.tile([128, 128], BF16, tag="sm", name="et_ps")
        nc.tensor.transpose(et_ps[:], ekT[:, tsl], iden_sb[:])
        nc.scalar.copy(eksc[:, i * 128 : (i + 1) * 128], et_ps[:])
        # --- cumsum over t (transposed): kcumT = ek_sc^T @ triu ---
        kcum_ps = psSm.tile([128, 128], F32, tag="sm", name="kcum_ps")
        nc.tensor.matmul(
            kcum_ps[:], lhsT=eksc[:, i * 128 : (i + 1) * 128],
            rhs=triu4_sb[:, 0:128], start=True, stop=True,
        )
        rt = rtp.tile([128, 128], BF16, tag="rt", name="rt")
        nc.scalar.activation(rt[:], kcum_ps[:], AF.Identity, bias=bias[:], scale=1.0)
        if i < NCH - 1:
            nbias = bip.tile([128, 1], F32, tag="bias", name="bias")
            nc.vector.tensor_add(nbias[:], bias[:], kcum_ps[:, 127:128])
            bias = nbias
        # --- qn = eq * 1/(kcum + carry + eps) * 1/S ---
        r2 = rtp.tile([128, 128], BF16, tag="r2", name="r2")
        nc.vector.reciprocal(r2[:], rt[:])
        q1 = rtp.tile([128, 128], BF16, tag="q1", name="q1")
        nc.vector.tensor_mul(q1[:], eqT[:, tsl], r2[:])
        nc.vector.tensor_mul(
            qnT[:, tsl], q1[:], Sinv[:, slot4 * 128 : (slot4 + 1) * 128]
        )
        # compact per-head copy of qn (matmul K operands must sit at base 0)
        qn4 = qnp.tile([16, 512], BF16, tag="qn4", name="qn4")
        for j in range(HC):
            nc.scalar.dma_start(
                qn4[0:16, j * 128 : (j + 1) * 128],
                qnT[32 * j : 32 * j + 16, tsl],
            )
        # --- at = ek_h^T qn_h, 4 heads into one [128, 512] PSUM ---
        at_ps = psA.tile([128, 512], F32, tag="psA", name="at_ps")
        for j in range(HC):
            nc.tensor.matmul(
                at_ps[:, j * 128 : (j + 1) * 128],
                lhsT=ek4[0:16, j * T + i * CH : j * T + (i + 1) * CH],
                rhs=qn4[0:16, j * 128 : (j + 1) * 128],
                start=True, stop=True,
            )
        at_sb = atp.tile([128, 512], BF16, tag="at", name="at_sb")
        nc.vector.tensor_mul(at_sb[:], at_ps[:], triu4_sb[:])
        # --- xo accumulation: v-term + kv-term ---
        for j in range(HC):
            p, h2 = j // 2, j % 2
            reg = xo_ps[p][:, 256 * h2 + slot2 * CH : 256 * h2 + (slot2 + 1) * CH]
            nc.tensor.matmul(
                reg,
                lhsT=v_sb[:, i * 256 + j * 64 : i * 256 + (j + 1) * 64],
                rhs=at_sb[:, j * 128 : (j + 1) * 128],
                start=True, stop=(i == 0),
            )
            if i > 0:
                nc.tensor.matmul(
                    reg,
                    lhsT=kvb[0:16, 64 * j : 64 * (j + 1)],
                    rhs=qn4[0:16, j * 128 : (j + 1) * 128],
                    start=False, stop=True,
                )
        # --- KV state update: per-head delta, fp32 accum + bf16 shadow ---
        if i < NCH - 1:
            E_ps = psSm.tile([16, 256], F32, tag="sm", name="E_ps")
            for j in range(HC):
                nc.tensor.matmul(
                    E_ps[0:16, j * 64 : (j + 1) * 64],
                    lhsT=eksc[:, i * 128 + 32 * j : i * 128 + 32 * j + 16],
                    rhs=v_sb[:, i * 256 + j * 64 : i * 256 + (j + 1) * 64],
                    start=True, stop=True,
                )
            nc.vector.tensor_add(kvf[:], kvf[:], E_ps[:])
            nkvb = kvp.tile([16, 256], BF16, tag="kv", name="kvb")
            nc.gpsimd.tensor_copy(out=nkvb[:], in_=kvf[:])
            kvb = nkvb
        # --- group end: stage xo, output projection, DMA out ---
        if slot2 == 1:
            for p in range(2):
                xoT = xop.tile([64, 512], BF16, tag=f"xoT{p}", name=f"xoT{p}")
                nc.scalar.copy(xoT[:], xo_ps[p][:])
                xoTs[p] = xoT
            for ck in range(2):
                tg = g2 * 2 + ck
                for half in range(2):
                    op_ps = psB.tile([128, 512], F32, tag="psB", name="op_ps")
                    for j in range(HC):
                        p, h2 = j // 2, j % 2
                        nc.tensor.matmul(
                            op_ps[:],
                            lhsT=xoTs[p][:, 256 * h2 + ck * CH : 256 * h2 + (ck + 1) * CH],
                            rhs=wout_sb[j][:, half * 512 : (half + 1) * 512],
                            start=(j == 0), stop=(j == 3),
                        )
                    ot = ostg.tile([128, 512], F16, tag="ot", name="ot")
                    if half == 0:
                        nc.vector.tensor_copy(out=ot[:], in_=op_ps[:])
                    else:
                        nc.scalar.copy(ot[:], op_ps[:])
                    nc.sync.dma_start(
                        out[tg * CH : (tg + 1) * CH, half * 512 : (half + 1) * 512],
                        ot[:],
                    )


def build(n_iter: int = 1):
    nc = bacc.Bacc("TRN2", target_bir_lowering=False, debug=False, num_devices=N_CORES)
    xT = nc.dram_tensor("xT", [D, T], BF16, kind="ExternalInput").ap()
    wqkT = nc.dram_tensor("wqkT", [D, 256], BF16, kind="ExternalInput").ap()
    wvT = nc.dram_tensor("wvT", [D, 256], BF16, kind="ExternalInput").ap()
    woutT = nc.dram_tensor("woutT", [256, 1024], BF16, kind="ExternalInput").ap()
    triu4 = nc.dram_tensor("triu4", [128, 512], BF16, kind="ExternalInput").ap()
    iden128 = nc.dram_tensor("iden128", [128, 128], BF16, kind="ExternalInput").ap()
    selhc = nc.dram_tensor("selhc", [128, 128], BF16, kind="ExternalInput").ap()
    out = nc.dram_tensor("partial", [T, D], F16, kind="ExternalOutput").ap()
    io = (xT, wqkT, wvT, woutT, triu4, iden128, selhc, out)

    with nc.allow_low_precision(
        reason="bf16 intermediates validated against fp32 reference"
    ), tile.TileContext(nc) as tc_, ExitStack() as ctx:
        if n_iter == 1:
            emit_body(nc, tc_, ctx, io)
        else:
            with tc_.For_i(0, n_iter, 1):
                with ExitStack() as inner:
                    emit_body(nc, tc_, inner, io)
    nc.compile()
    return nc


def make_in_maps(x, w_qkv, w_out, fc_code):
    x = np.asarray(x, dtype=np.float32)
    w_qkv = np.asarray(w_qkv, dtype=np.float32)
    w_out = np.asarray(w_out, dtype=np.float32)
    fc_code = np.asarray(fc_code, dtype=np.float32)
    code = fc_code[0]  # (16, 16, 64)

    triu = np.triu(np.ones((128, 128), dtype=np.float32))
    triu4 = np.tile(triu, (1, 4)).astype(BFNP)
    iden128 = np.eye(128, dtype=np.float32).astype(BFNP)
    selhc = np.zeros((128, 128), dtype=np.float32)
    for h in range(HC):
        selhc[32 * h : 32 * h + 16, 32 * h : 32 * h + 32] = 1.0
    selhc = selhc.astype(BFNP)
    xTs = [np.ascontiguousarray(x[b].T).astype(BFNP) for b in range(B)]

    in_maps = []
    for core in range(N_CORES):
        b, g = core // HC, core % HC
        hs = [g * HC + j for j in range(HC)]
        wqk = np.zeros((256, D), dtype=np.float32)
        for j, h in enumerate(hs):
            wq_eff = code[h] @ w_qkv[h * HD : (h + 1) * HD]  # (16, 1024)
            wk_eff = code[h] @ w_qkv[D + h * HD : D + (h + 1) * HD]
            wqk[32 * j : 32 * j + 16] = wq_eff
            wqk[128 + 32 * j : 128 + 32 * j + 16] = wk_eff
        wqkT = np.ascontiguousarray(wqk.T).astype(BFNP)  # (1024, 256)
        wvT = np.ascontiguousarray(
            np.concatenate(
                [w_qkv[2 * D + h * HD : 2 * D + (h + 1) * HD] for h in hs], axis=0
            ).T
        ).astype(BFNP)  # (1024, 256)
        woutT = (
            np.ascontiguousarray(
                np.concatenate([w_out[:, h * HD : (h + 1) * HD].T for h in hs], axis=0)
            )
            * np.float32(SCALE)
        ).astype(BFNP)  # (256, 1024)
        in_maps.append(
            {
                "xT": xTs[b],
                "wqkT": wqkT,
                "wvT": wvT,
                "woutT": woutT,
                "triu4": triu4,
                "iden128": iden128,
                "selhc": selhc,
            }
        )
    return in_maps


def gather(results):
    out = np.zeros((B, T, D), dtype=np.float32)
    for core in range(N_CORES):
        out[core // HC] += results[core]["partial"].astype(np.float32)
    return out


_NC_CACHE = {}


def kernel(x, w_qkv, w_out, fc_code):
    from concourse.bass_utils import run_bass_kernel_spmd

    if 1 not in _NC_CACHE:
        _NC_CACHE[1] = build(1)
    nc = _NC_CACHE[1]
    in_maps = make_in_maps(x, w_qkv, w_out, fc_code)
    res = run_bass_kernel_spmd(nc, in_maps, list(range(N_CORES)))
    return gather(res.results)
